# revision 48
# baseline (speedup 1.0000x reference)
"""Transformer block (B=4,T=2048,C=1024,H=16) on 8 trn2 cores, zero-communication.

Split: core c -> sequence b=c//2, token parity s=c%2. Each core computes the
full block output for its 1024 strided query tokens (positions s, s+2, ...),
recomputing LN1+K/V for the whole 2048-token context locally (no collectives).
The program is identical on all cores; parity enters only through staged data
(xq rows, boundary mask values).

Layouts ("T" suffix = [feature_partitions, token_free]):
  nT  [1024c, T]   ln1(x) transposed, bf16
  kT  [1024d, T]   keys transposed (head-pair per 128-row tile), bf16
  vA  [128tok, 16*65] values + ones column per head, bf16
  qT  [1024d, own] queries transposed (scaled 1/8 host-side), bf16
  scoresT psum [128kv, q] -> exp -> expT bf16 -> av psum [65, q] (row 64 = sum)
  attnT [1024d, own] normalized attention out transposed, bf16
  then c_proj -> +xq -> ln2 -> mT -> fc/gelu -> hT -> proj -> +x2 -> out.
"""
import sys

sys.path.insert(0, "/opt/trn_rl_repo")

import numpy as np
import ml_dtypes

import concourse.bass as bass
import concourse.mybir as mybir
import concourse.tile as tile
from concourse import bacc
from concourse.masks import make_identity

FP32 = mybir.dt.float32
BF16 = mybir.dt.bfloat16
AF = mybir.ActivationFunctionType
ALU = mybir.AluOpType

C = 1024
H = 16
HS = 64
FF = 4096
LN_EPS = 1e-5
P = 128


def build_nc(T=2048):
    own = T // 2          # query tokens per core
    NKV = T // P          # kv token tiles
    QM = min(512, own)    # q-macro width (compact q indices)
    NM = own // QM        # macros
    NQT = own // P        # own-token tiles
    NCT = C // P          # feature tiles (8)

    nc = bacc.Bacc(None, target_bir_lowering=False, debug=False)

    xc = nc.dram_tensor("xc", [T, C], BF16, kind="ExternalInput")
    xq = nc.dram_tensor("xq", [own, C], BF16, kind="ExternalInput")
    wq = nc.dram_tensor("wq", [C, C], BF16, kind="ExternalInput")
    wk = nc.dram_tensor("wk", [C, C], BF16, kind="ExternalInput")
    wv = nc.dram_tensor("wv", [C, C], BF16, kind="ExternalInput")
    wc = nc.dram_tensor("wc", [C, C], BF16, kind="ExternalInput")
    wf = nc.dram_tensor("wf", [C, FF], BF16, kind="ExternalInput")
    wp = nc.dram_tensor("wp", [FF, C], BF16, kind="ExternalInput")
    msk = nc.dram_tensor("msk", [P, P], BF16, kind="ExternalInput")
    yout = nc.dram_tensor("yout", [own, C], FP32, kind="ExternalOutput")

    with tile.TileContext(nc) as tc:
        import contextlib

        with contextlib.ExitStack() as ctx:
            const = ctx.enter_context(tc.tile_pool(name="const", bufs=1))
            xin = ctx.enter_context(tc.tile_pool(name="xin", bufs=4))
            lnp = ctx.enter_context(tc.tile_pool(name="lnp", bufs=3))
            nbp = ctx.enter_context(tc.tile_pool(name="nbp", bufs=2))
            # 16KB/part slots x2: nT super-tiles -> hT super-tiles
            g1 = ctx.enter_context(tc.tile_pool(name="g1", bufs=2))
            # 4KB/part slots: kT -> wf halves -> wp halves
            g2 = ctx.enter_context(tc.tile_pool(name="g2", bufs=NCT))
            # 4KB/part slots: nqT -> x2
            g3 = ctx.enter_context(tc.tile_pool(name="g3", bufs=NCT))
            # ~2KB/part slots x16: vA -> xr pieces -> mT
            vap = ctx.enter_context(tc.tile_pool(name="vap", bufs=max(NKV, 16)))
            # 2KB/part slots x10: qT -> wc -> wp high tiles
            qwp = ctx.enter_context(tc.tile_pool(name="qwp", bufs=NCT))
            # 2KB/part slots x8: attnT -> yout staging
            atp = ctx.enter_context(tc.tile_pool(name="atp", bufs=NCT))
            exq = ctx.enter_context(tc.tile_pool(name="exq", bufs=4))    # expT pairs
            smp = ctx.enter_context(tc.tile_pool(name="smp", bufs=3))    # softmax small
            wsp = ctx.enter_context(tc.tile_pool(name="wsp", bufs=4))    # wq/wk 1024-col strips
            wvp = ctx.enter_context(tc.tile_pool(name="wvp", bufs=9))    # wv halves

            ps_sc = ctx.enter_context(tc.tile_pool(name="ps_sc", bufs=2, space="PSUM"))
            ps_av = ctx.enter_context(tc.tile_pool(name="ps_av", bufs=2, space="PSUM"))
            ps_a = ps_sc  # accumulation chains share the 2-bank "sc" slots

            ident = const.tile([P, P], BF16)
            make_identity(nc, ident)
            ones1 = const.tile([1, 64], BF16)
            nc.vector.memset(ones1, 1.0)
            epst = const.tile([P, 1], FP32)
            nc.vector.memset(epst, LN_EPS)
            maskt = const.tile([P, P], BF16)
            nc.sync.dma_start(maskt[:], msk[:])

            # PE warmup: real matmuls (transpose-mode doesn't register as
            # PE-busy for the HAM clock gate) while the first LN chunks
            # stream in.
            for wi in range(96):
                wps = ps_av.tile([P, P], FP32, tag="av", name=f"warm{wi}")
                nc.tensor.matmul(wps[:], ident[:], ident[:], start=True, stop=True)

            def layer_norm_to_bf16(xt, out_bf, uid):
                """xt [128, C] f32 -> out_bf [128, C] bf16 (normalized)."""
                stats = lnp.tile([P, 2, 6], FP32, tag="stats", name=f"st{uid}")
                nc.vector.bn_stats(stats[:, 0, :], xt[:, 0:512])
                nc.vector.bn_stats(stats[:, 1, :], xt[:, 512:1024])
                mv = lnp.tile([P, 2], FP32, tag="mv", name=f"mv{uid}")
                nc.vector.bn_aggr(mv[:], stats[:])
                sd = lnp.tile([P, 1], FP32, tag="sd", name=f"sd{uid}")
                nc.scalar.activation(sd[:], mv[:, 1:2], AF.Sqrt, bias=epst[:])
                rs = lnp.tile([P, 1], FP32, tag="rs", name=f"rs{uid}")
                nc.vector.reciprocal(rs[:], sd[:])
                nc.vector.tensor_scalar(
                    out=out_bf[:],
                    in0=xt[:],
                    scalar1=mv[:, 0:1],
                    scalar2=rs[:],
                    op0=ALU.subtract,
                    op1=ALU.mult,
                )

            _sc = nc.enter_named_scope("ph_ln1", False)[0]
            # ---- Phase A: load x, LN1, transpose -> nT, fused V(oj=0) ----
            # vA(oj=0) is computed per context tile as soon as its LN lands:
            # real matmuls from t~=5us keep the HAM clock warm through the
            # DMA/DVE-bound LN phase, and the standalone vA phase disappears.
            nTb = [g1.tile([P, 4 * T], BF16, tag="g1", name=f"nTb{i}") for i in range(2)]
            nT = [nTb[i // 4][:, (i % 4) * T : (i % 4 + 1) * T] for i in range(NCT)]
            kT = [g2.tile([P, T], BF16, tag="g2", name=f"kT{i}") for i in range(NCT)]
            NTM = T // 512

            vA = []
            for tt in range(NKV):
                v = vap.tile([P, H * 65], BF16, tag="va", name=f"vA{tt}")
                v3 = v.rearrange("p (h k) -> p h k", k=65)
                nc.vector.memset(v3[:, :, 64:65], 1.0)
                vA.append(v)
            wvhs = {}
            for oj in range(2):
                wvh = []
                for ci in range(NCT):
                    w = wvp.tile([P, 512], BF16, tag="v", name=f"wv{oj}_{ci}")
                    nc.sync.dma_start(
                        w[:], wv[P * ci : P * (ci + 1), 512 * oj : 512 * (oj + 1)]
                    )
                    wvh.append(w)
                wvhs[oj] = wvh

            def emit_v(oj, tt, in_attn):
                tag = "av" if in_attn else "sc"
                pool = ps_av if in_attn else ps_a
                ps = pool.tile([P, 512], FP32, tag=tag, name=f"vps{oj}_{tt}")
                for ci in range(NCT):
                    nc.tensor.matmul(
                        ps[:],
                        nT[ci][:, P * tt : P * (tt + 1)],
                        wvhs[oj][ci][:],
                        start=(ci == 0),
                        stop=(ci == NCT - 1),
                    )
                v3 = vA[tt].rearrange("p (h k) -> p h k", k=65)
                ps3 = ps.rearrange("p (h k) -> p h k", k=64)
                nc.vector.tensor_copy(v3[:, 8 * oj : 8 * (oj + 1), 0:64], ps3[:])

            # one strided DMA brings all 8 contraction blocks of a 128-col
            # weight strip: [C, 128] -> [128p, 8ci, 128]; per-block triggers
            # (~600ns each on the sync queue) were saturating it.
            wq3 = wq.rearrange("(c p) n -> p c n", p=P)
            wk3 = wk.rearrange("(c p) n -> p c n", p=P)

            def load_wstrip(src3, ot, name):
                w = wsp.tile([P, NCT, P], BF16, tag="qk", name=name)
                nc.sync.dma_start(w[:], src3[:, :, P * ot : P * (ot + 1)])
                return w

            def emit_kT(ot, in_attn):
                wkb = load_wstrip(wk3, ot, f"wk{ot}")
                for tm in range(NTM):
                    tag = "av" if in_attn else "sc"
                    pool = ps_av if in_attn else ps_a
                    ps = pool.tile([P, 512], FP32, tag=tag, name=f"kps{ot}_{tm}")
                    for ci in range(NCT):
                        nc.tensor.matmul(
                            ps[:],
                            wkb[:, ci, :],
                            nT[ci][:, 512 * tm : 512 * (tm + 1)],
                            start=(ci == 0),
                            stop=(ci == NCT - 1),
                        )
                    nc.vector.tensor_copy(
                        kT[ot][:, 512 * tm : 512 * (tm + 1)], ps[:]
                    )

            def ln_transpose_tile(dst_list, dst_col, nb):
                # copies mostly on ScalarE: VectorE is the floor of the LN
                # phase (bn_stats/tensor_scalar) while ScalarE idles
                for ct in range(NCT):
                    pst = ps_av.tile([P, P], BF16, tag="av", name=f"tr{dst_col}_{ct}")
                    nc.tensor.transpose(pst[:], nb[:, P * ct : P * (ct + 1)], ident[:])
                    dst = dst_list[ct][:, P * dst_col : P * (dst_col + 1)]
                    if ct == NCT - 1:
                        nc.vector.tensor_copy(dst, pst[:])
                    else:
                        nc.scalar.activation(dst, pst[:], AF.Copy)

            def warm_sprinkle(n, uid):
                # tiny real matmuls to keep the HAM clock at 8/8 through
                # transpose/DVE-heavy stretches (transposes don't count)
                for wi in range(n):
                    wps = ps_av.tile([P, P], FP32, tag="av", name=f"ws{uid}_{wi}")
                    nc.tensor.matmul(wps[:], ident[:], ident[:], start=True, stop=True)

            nqT = [g3.tile([P, own], BF16, tag="g3", name=f"nqT{i}") for i in range(NCT)]
            qT = [qwp.tile([P, own], BF16, tag="qw", name=f"qT{i}") for i in range(NCT)]

            def ctx_tile(kt):
                xt = xin.tile([P, C], BF16, tag="xt", name=f"x{kt}")
                nc.sync.dma_start(xt[:], xc[P * kt : P * (kt + 1), :])
                nb = nbp.tile([P, C], BF16, tag="nb", name=f"nb{kt}")
                layer_norm_to_bf16(xt, nb, f"a{kt}")
                # 4 transposes packed into one PSUM bank -> one wide copy:
                # avoids the per-transpose slot round-trip that stalls the
                # PE queue (and cools the HAM clock) in the LN-bound front
                for g in range(2):
                    pst = ps_av.tile([P, 4, P], BF16, tag="av", name=f"trp{kt}_{g}")
                    for i in range(4):
                        ct = 4 * g + i
                        nc.tensor.transpose(
                            pst[:, i, :], nb[:, P * ct : P * (ct + 1)], ident[:]
                        )
                    dstv = nTb[g].rearrange("p (f t) -> p f t", f=4)
                    nc.scalar.activation(
                        dstv[:, :, P * kt : P * (kt + 1)], pst[:], AF.Copy
                    )
                if kt >= 1:
                    emit_v(0, kt - 1, False)

            def own_tile(qt):
                xt = xin.tile([P, C], BF16, tag="xt", name=f"xq{qt}")
                nc.sync.dma_start(xt[:], xq[P * qt : P * (qt + 1), :])
                nb = nbp.tile([P, C], BF16, tag="nb", name=f"nq{qt}")
                layer_norm_to_bf16(xt, nb, f"q{qt}")
                ln_transpose_tile(nqT, qt, nb)
                warm_sprinkle(2, f"q{qt}")

            def emit_qT(ot, m, in_attn):
                wqb = load_wstrip(wq3, ot, f"wq{ot}_{m}")
                tag = "av" if in_attn else "sc"
                pool = ps_av if in_attn else ps_a
                ps = pool.tile([P, QM], FP32, tag=tag, name=f"qps{ot}_{m}")
                for ci in range(NCT):
                    nc.tensor.matmul(
                        ps[:],
                        wqb[:, ci, :],
                        nqT[ci][:, QM * m : QM * (m + 1)],
                        start=(ci == 0),
                        stop=(ci == NCT - 1),
                    )
                nc.vector.tensor_copy(qT[ot][:, QM * m : QM * (m + 1)], ps[:])

            # ---- pre-attention schedule: LN tiles / V / Q interleaved ----
            # front: elementwise-bound LN pipeline with vA matmuls keeping
            # the PE warm; tail: PE-dense qT chains + first two kT tiles.
            for kt in range(4):
                ctx_tile(kt)
            for i in range(NQT):
                ctx_tile(4 + i)
                own_tile(i)
            for kt in range(12, NKV):
                ctx_tile(kt)
                for ot4 in range(4):
                    om = 4 * (kt - 12) + ot4
                    emit_qT(om % NCT, om // NCT, False)
            emit_v(0, NKV - 1, False)
            for ot in range(4):
                emit_kT(ot, False)

            # remaining kT tiles (per-512-token sub-chain) and vA(oj=1)
            # tiles stream into the attention phase as PE fillers, ordered
            # so each arrives before its first consumer.
            def filler_gen():
                ks = [("k", ot, tm) for ot in range(4, NCT) for tm in range(NTM)]
                vs = [("v", 1, tt) for tt in range(NKV)]
                ki = vi = 0
                while ki < len(ks) or vi < len(vs):
                    if ki < len(ks):
                        yield ks[ki]
                        ki += 1
                    if vi < len(vs):
                        yield vs[vi]
                        vi += 1

            _fill = filler_gen()
            kT_done = {ot: set(range(NTM)) if ot < 4 else set() for ot in range(NCT)}
            emitted_v1 = set()
            _wkb_cache = {}

            def emit_kT_tm(ot, tm):
                if ot not in _wkb_cache:
                    _wkb_cache[ot] = load_wstrip(wk3, ot, f"wkf{ot}")
                wkb = _wkb_cache[ot]
                ps = ps_av.tile([P, 512], FP32, tag="av", name=f"kfps{ot}_{tm}")
                for ci in range(NCT):
                    nc.tensor.matmul(
                        ps[:],
                        wkb[:, ci, :],
                        nT[ci][:, 512 * tm : 512 * (tm + 1)],
                        start=(ci == 0),
                        stop=(ci == NCT - 1),
                    )
                nc.vector.tensor_copy(kT[ot][:, 512 * tm : 512 * (tm + 1)], ps[:])
                kT_done[ot].add(tm)

            def _emit_one(it):
                if it[0] == "k":
                    emit_kT_tm(it[1], it[2])
                else:
                    emitted_v1.add(it[2])
                    emit_v(it[1], it[2], True)

            _warm = [0]

            def emit_filler(n):
                for _ in range(n):
                    it = next(_fill, None)
                    if it is None:
                        # out of real work: cheap matmuls keep the HAM clock
                        # warm through the PE-light tail heads
                        _warm[0] += 1
                        wps = ps_av.tile(
                            [P, P], FP32, tag="av", name=f"aw{_warm[0]}"
                        )
                        nc.tensor.matmul(
                            wps[:], ident[:], ident[:], start=True, stop=True
                        )
                        return
                    _emit_one(it)

            def require(hp, m):
                while len(kT_done[hp]) < NTM:
                    _emit_one(next(_fill))
                if hp >= H // 4:
                    jm = 2 * QM * (m + 1) // P
                    while not all(tt in emitted_v1 for tt in range(jm)):
                        _emit_one(next(_fill))

            nc.leave_named_scope("ph_ln1", _sc, False)
            _sc = nc.enter_named_scope("ph_attn", False)[0]
            # ---- Phase D: attention --------------------------------------
            attnT = [
                atp.tile([P, own], BF16, tag="at", name=f"attnT{i}") for i in range(NCT)
            ]
            def emit_normalize(h, m, avp):
                # copy the raw head output + sum row out first: avp's PSUM
                # bank frees after two cheap copies instead of after the
                # whole bcast->reciprocal->mul chain, so the next macro's AV
                # accumulation starts ~2us earlier. The divide then runs
                # in-place on SBUF off the critical path.
                hp, r = h // 2, h % 2
                dst = attnT[hp][64 * r : 64 * r + 64, QM * m : QM * (m + 1)]
                srow = smp.tile([1, QM], BF16, tag="rc", name=f"sr{h}_{m}")
                tmp = smp.tile([64, QM], BF16, tag="rc", name=f"tm{h}_{m}")
                with nc.allow_low_precision(reason="softmax sum row to bf16"):
                    nc.vector.tensor_copy(srow[:], avp[64:65, :])
                    nc.vector.tensor_copy(tmp[:], avp[0:64, :])
                bcp = ps_av.tile([64, QM], FP32, tag="av", name=f"bc{h}_{m}")
                nc.tensor.matmul(
                    bcp[:], ones1[0:1, :], srow[0:1, :], start=True, stop=True
                )
                bcs = smp.tile([64, QM], FP32, tag="bc", name=f"bcs{h}_{m}")
                nc.vector.reciprocal_approx_fast(bcs[:], bcp[:])
                nc.vector.tensor_mul(dst, tmp[:], bcs[:])

            # software-pipelined macros: each macro's tail (last two AV
            # accumulations + normalize) is deferred into the NEXT macro's
            # j-loop so the next exp never queues behind it on the in-order
            # PE queue; require() prefetches one macro ahead for the same
            # reason.
            macros = [(hp, m) for hp in range(H // 2) for m in range(NM)]
            pending = [None, None]

            def flush_avs():
                if pending[0] is not None:
                    pending[0]()
                    pending[0] = None

            def flush_norm():
                if pending[1] is not None:
                    pending[1]()
                    pending[1] = None

            require(*macros[0])
            for idx, (hp, m) in enumerate(macros):
                jmax = 2 * QM * (m + 1) // P
                avp = [
                    ps_av.tile([65, QM], FP32, tag="avp", name=f"av{hp}_{m}_{r}")
                    for r in range(2)
                ]
                exs = {}

                def emit_av(j, r, avp=avp, exs=exs, hp=hp, jmax=jmax):
                    ex, w0 = exs[(j, r)]
                    nc.tensor.matmul(
                        avp[r][:, w0:QM],
                        vA[j][:, 65 * (2 * hp + r) : 65 * (2 * hp + r) + 65],
                        ex[:, QM * r + w0 : QM * (r + 1)],
                        start=(j == 0),
                        stop=(j == jmax - 1),
                    )
                    if r == 1:
                        del exs[(j, 0)], exs[(j, 1)]

                scs = {}

                def emit_scores(j, hp=hp, m=m, scs=scs):
                    wq_ = max(0, (P * j - 2 * QM * m) // 2)
                    diag = P * j >= 2 * QM * m
                    sc = ps_sc.tile(
                        [P, 2 * QM], FP32, tag="sc", name=f"sc{hp}_{m}_{j}"
                    )
                    for r in range(2):
                        if diag:
                            nc.tensor.matmul(
                                sc[:, QM * r + wq_ : QM * r + wq_ + 64],
                                maskt[64 * r : 64 * r + 64, :],
                                ident[64 * r : 64 * r + 64, 64 * r : 64 * r + 64],
                                start=True,
                                stop=False,
                            )
                        nc.tensor.matmul(
                            sc[:, QM * r + wq_ : QM * (r + 1)],
                            kT[hp][64 * r : 64 * r + 64, P * j : P * (j + 1)],
                            qT[hp][
                                64 * r : 64 * r + 64, QM * m + wq_ : QM * (m + 1)
                            ],
                            start=not diag,
                            stop=True,
                        )
                    scs[j] = (sc, wq_)

                def emit_exp(j, hp=hp, m=m, scs=scs, exs=exs):
                    sc, wq_ = scs.pop(j)
                    ex = exq.tile(
                        [P, 2 * QM], BF16, tag="ex", name=f"ex{hp}_{m}_{j}"
                    )
                    sc3 = sc.rearrange("p (r q) -> p r q", r=2)
                    ex3 = ex.rearrange("p (r q) -> p r q", r=2)
                    nc.scalar.activation(
                        ex3[:, :, wq_:QM], sc3[:, :, wq_:QM], AF.Exp
                    )
                    exs[(j, 0)] = (ex, wq_)
                    exs[(j, 1)] = (ex, wq_)

                # scores run one kv-tile ahead of exp so the next exp never
                # queues behind AV/filler matmuls on the in-order PE queue
                emit_scores(0)
                for j in range(jmax):
                    emit_exp(j)
                    if j + 1 < jmax:
                        emit_scores(j + 1)
                    if j == 0:
                        flush_avs()
                    if j == 1:
                        flush_norm()
                    if j >= 2:
                        emit_av(j - 2, 0)
                        emit_av(j - 2, 1)
                    if j == 3 and idx + 1 < len(macros):
                        require(*macros[idx + 1])
                    if j % 3 == 2:
                        emit_filler(1)

                def tail_avs(emit_av=emit_av, jmax=jmax):
                    for jt in range(max(0, jmax - 2), jmax):
                        emit_av(jt, 0)
                        emit_av(jt, 1)

                def tail_norm(avp=avp, hp=hp, m=m):
                    emit_normalize(2 * hp, m, avp[0])
                    emit_normalize(2 * hp + 1, m, avp[1])

                pending[0] = tail_avs
                pending[1] = tail_norm

            flush_avs()
            flush_norm()
            emit_filler(10**6)
            nc.leave_named_scope("ph_attn", _sc, False)
            _sc = nc.enter_named_scope("ph_cproj", False)[0]
            # ---- Phase E: c_proj + residual ------------------------------
            wcb = []
            for ci in range(NCT):
                w = qwp.tile([P, C], BF16, tag="qw", name=f"wc{ci}")
                nc.sync.dma_start(w[:], wc[P * ci : P * (ci + 1), :])
                wcb.append(w)
            x2 = []
            for qt in range(NQT):
                xx = g3.tile([P, C], FP32, tag="g3", name=f"x2_{qt}")
                for oj in range(2):
                    xr = vap.tile([P, 512], BF16, tag="va", name=f"xr{qt}_{oj}")
                    nc.sync.dma_start(
                        xr[:], xq[P * qt : P * (qt + 1), 512 * oj : 512 * (oj + 1)]
                    )
                    ps = ps_a.tile([P, 512], FP32, tag="sc", name=f"cps{qt}_{oj}")
                    for ci in range(NCT):
                        nc.tensor.matmul(
                            ps[:],
                            attnT[ci][:, P * qt : P * (qt + 1)],
                            wcb[ci][:, 512 * oj : 512 * (oj + 1)],
                            start=(ci == 0),
                            stop=(ci == NCT - 1),
                        )
                    nc.vector.tensor_add(
                        xx[:, 512 * oj : 512 * (oj + 1)],
                        ps[:],
                        xr[:],
                    )
                x2.append(xx)

            nc.leave_named_scope("ph_cproj", _sc, False)
            _sc = nc.enter_named_scope("ph_ln2", False)[0]
            # ---- Phase F: LN2 -> mT --------------------------------------
            mT = [vap.tile([P, own], BF16, tag="va", name=f"mT{i}") for i in range(NCT)]
            for qt in range(NQT):
                mb = nbp.tile([P, C], BF16, tag="nb", name=f"mb{qt}")
                layer_norm_to_bf16(x2[qt], mb, f"m{qt}")
                ln_transpose_tile(mT, qt, mb)

            nc.leave_named_scope("ph_ln2", _sc, False)
            _sc = nc.enter_named_scope("ph_mlp", False)[0]
            # ---- Phases G+H: MLP per f-half ------------------------------
            NMQ = own // QM
            for fh in range(2):
                wfh = []
                for ci in range(NCT):
                    w = g2.tile([P, 2048], BF16, tag="g2", name=f"wf{fh}_{ci}")
                    nc.sync.dma_start(
                        w[:], wf[P * ci : P * (ci + 1), 2048 * fh : 2048 * (fh + 1)]
                    )
                    wfh.append(w)
                hTb = [
                    g1.tile([P, 4 * 2 * own], BF16, tag="g1", name=f"hTb{fh}_{i}")
                    for i in range(2)
                ]

                def h_slice(ftl, lo, sz):
                    base = 2 * own * ((ftl // 2) % 4) + own * (ftl % 2)
                    return hTb[ftl // 8][:, base + lo : base + lo + sz]

                for ftl in range(16):
                    for mq in range(NMQ):
                        ps = ps_a.tile(
                            [P, QM], FP32, tag="sc", name=f"fps{fh}_{ftl}_{mq}"
                        )
                        for ci in range(NCT):
                            nc.tensor.matmul(
                                ps[:],
                                wfh[ci][:, P * ftl : P * (ftl + 1)],
                                mT[ci][:, QM * mq : QM * (mq + 1)],
                                start=(ci == 0),
                                stop=(ci == NCT - 1),
                            )
                        nc.scalar.activation(
                            h_slice(ftl, QM * mq, QM), ps[:], AF.Gelu_apprx_tanh
                        )
                # wp row-tiles live in pools whose previous tenants free up
                # during/before fc, so the DMA streams in behind the fc
                # compute instead of stalling the fc->proj boundary.
                wph = []
                for t16 in range(16):
                    pool, tg = (vap, "va") if t16 < 8 else (qwp, "qw")
                    w = pool.tile([P, 1024], BF16, tag=tg, name=f"wp{fh}_{t16}")
                    r0 = 2048 * fh + P * t16
                    nc.sync.dma_start(w[:], wp[r0 : r0 + P, :])
                    wph.append(w)
                for qt in range(NQT):
                    for oj in range(2):
                        ps = ps_a.tile(
                            [P, 512], FP32, tag="sc", name=f"pps{fh}_{qt}_{oj}"
                        )
                        for ftl in range(16):
                            nc.tensor.matmul(
                                ps[:],
                                h_slice(ftl, P * qt, P),
                                wph[ftl][:, 512 * oj : 512 * (oj + 1)],
                                start=(ftl == 0),
                                stop=(ftl == 15),
                            )
                        if fh == 0:
                            nc.vector.tensor_add(
                                x2[qt][:, 512 * oj : 512 * (oj + 1)],
                                ps[:],
                                x2[qt][:, 512 * oj : 512 * (oj + 1)],
                            )
                        else:
                            yo = atp.tile([P, 512], FP32, tag="at", name=f"yo{qt}_{oj}")
                            nc.vector.tensor_add(
                                yo[:], ps[:], x2[qt][:, 512 * oj : 512 * (oj + 1)]
                            )
                            nc.sync.dma_start(
                                yout[P * qt : P * (qt + 1), 512 * oj : 512 * (oj + 1)],
                                yo[:],
                            )
            nc.leave_named_scope("ph_mlp", _sc, False)

    nc.compile()
    return nc


def stage_inputs(x, c_attn_w, c_proj_w, fc_w, proj_w, ln1_g, ln2_g, T=2048, n_cores=8):
    """Host-side prep: per-core input maps. x: (B, T, C) f32."""
    bf = ml_dtypes.bfloat16
    g1w = c_attn_w * ln1_g[:, None]
    wqh = np.ascontiguousarray((g1w[:, 0:C] * 0.125).astype(bf))
    wkh = np.ascontiguousarray(g1w[:, C : 2 * C].astype(bf))
    wvh = np.ascontiguousarray(g1w[:, 2 * C : 3 * C].astype(bf))
    wch = np.ascontiguousarray(c_proj_w.astype(bf))
    wfh = np.ascontiguousarray((fc_w * ln2_g[:, None]).astype(bf))
    wph = np.ascontiguousarray(proj_w.astype(bf))
    in_maps = []
    for c in range(n_cores):
        b, s = c // 2, c % 2
        xcv = np.ascontiguousarray(x[b][:T].astype(bf))
        xqv = np.ascontiguousarray(x[b][s:T:2].astype(bf))
        ul = np.arange(64)[:, None]
        kvl = np.arange(P)[None, :]
        mask = np.where(kvl > 2 * ul + s, -300.0, 0.0).astype(np.float32)
        mask = np.concatenate([mask, mask], axis=0)
        in_maps.append(
            {
                "xc": xcv,
                "xq": xqv,
                "wq": wqh,
                "wk": wkh,
                "wv": wvh,
                "wc": wch,
                "wf": wfh,
                "wp": wph,
                "msk": mask.astype(bf),
            }
        )
    return in_maps




_NC_CACHE = {}


def _get_nc(T=2048):
    if T not in _NC_CACHE:
        _NC_CACHE[T] = build_nc(T=T)
    return _NC_CACHE[T]


def kernel(**inputs):
    """Full transformer block on 8 NeuronCores. Takes/returns full numpy arrays."""
    from concourse.bass_utils import run_bass_kernel_spmd

    x = np.asarray(inputs["x"], dtype=np.float32)
    B, T, C_ = x.shape
    nc = _get_nc(T=T)
    in_maps = stage_inputs(
        x,
        np.asarray(inputs["c_attn_w"], dtype=np.float32),
        np.asarray(inputs["c_proj_w"], dtype=np.float32),
        np.asarray(inputs["fc_w"], dtype=np.float32),
        np.asarray(inputs["proj_w"], dtype=np.float32),
        np.asarray(inputs["ln1_g"], dtype=np.float32),
        np.asarray(inputs["ln2_g"], dtype=np.float32),
        T=T,
        n_cores=8,
    )
    res = run_bass_kernel_spmd(nc, in_maps, list(range(8)))
    out = np.empty((B, T, C_), dtype=np.float32)
    for c in range(8):
        b, s = c // 2, c % 2
        out[b, s::2, :] = res.results[c]["yout"]
    return out

